# revision 6
# baseline (speedup 1.0000x reference)
"""GAT-style 2-layer GNN message passing on 8 Trainium2 NeuronCores.

Math note: for this reference, the segment-softmax ratio
  num/den = (sum_j h[j]*exp((s_l[i]+s_r[j])/2d)) / (sum_j exp((s_l[i]+s_r[j])/2d))
has the destination factor exp(s_l[i]/2d) cancel, so per layer we only need
  a[i] = (sum_{j in N(i)} w_j*h_j) / (sum_{j in N(i)} w_j),  w_j = exp(s_r[j]/2d).

Sharding: nodes split into 8 contiguous destination ranges (6250/core).
Each core builds table rows [g=w*h (256) | w (4) | pad] (bf16, 768B) for its
own nodes; the table is split in two halves by local row (A: 0..3199,
B: 3200..6249) and distributed with two pipelined AllGathers per layer.
Each core aggregates its own destinations: per-edge dma_gather of source
rows (HBM row order sorted ascending per gather for locality, per-block
chunk-exact counts with -1 tail trim), then one-hot matmul segment-sum into
PSUM. A-half partial sums spill to an SBUF accumulator so the A stream can
run ahead while the B-half AllGather is still in flight; B-half partials
merge with the accumulator at divide time. Layer-1 table rows are built
inline in the layer-0 aggregation sweep (the x1 transpose for the layer-1
GEMM goes through a DRAM DMA-transpose round trip).
"""

import os
import sys

import numpy as np
import ml_dtypes

sys.path.insert(0, "/opt/trn_rl_repo")

import concourse.bacc as bacc
import concourse.bass as bass
import concourse.mybir as mybir
import concourse.tile as tile
from concourse.bass_utils import run_bass_kernel_spmd

BF16 = mybir.dt.bfloat16
F32 = mybir.dt.float32
I16 = mybir.dt.int16

N, DIN, E = 50000, 128, 800000
H, D = 4, 64
F = H * D  # 256
FH = F + H  # 260
NCORE = 8
NPC = N // NCORE  # 6250
NBLK = (NPC + 127) // 128  # 49 destination blocks per core
EPS = 1e-5
SLOPE = 0.01
ROWE = 384  # table row: 256 g + 4 w + 124 pad (bf16) = 768 bytes
SPLITA = 3200  # local rows 0..3199 -> table A (25 blocks), rest -> table B
SPLITB = NPC - SPLITA  # 3050
NBLKA = SPLITA // 128  # 25
K1 = (1 + SLOPE) / 2 / (2 * D)
K2 = (1 - SLOPE) / 2 / (2 * D)

LAST_RESULTS = None

AF = mybir.ActivationFunctionType
ALU = mybir.AluOpType


def _host_prep(x, edge_index, W0, b0, W1, b1, att0, att1, gamma, beta):
    """Build all per-core and shared input arrays.

    Per (core, block, half) the edge list is sorted by source table row.
    All cores share one static chunk count per (block, half): the max over
    cores. Cores short of (nch-1)*128+1 edges are topped up with row-0
    dummy edges (one-hot zero), then -1-padded to nch*128 so the gather
    ucode trims the tail to the top-up count on every core (keeping the
    descriptor count consistent with the ring reservation).
    """
    bf16 = ml_dtypes.bfloat16
    dst = np.asarray(edge_index[0], dtype=np.int64)
    src = np.asarray(edge_index[1], dtype=np.int64)

    plain_ln = bool(
        np.allclose(np.asarray(gamma), 1.0) and np.allclose(np.asarray(beta), 0.0)
    )

    per_core = []  # [c][b][half] -> (rows_sorted, lb_sorted)
    for c in range(NCORE):
        m = (dst >= c * NPC) & (dst < (c + 1) * NPC)
        ld = dst[m] - c * NPC
        s = src[m]
        owner = s // NPC
        srow = s - owner * NPC
        inA = srow < SPLITA
        idxA_all = owner * SPLITA + srow
        idxB_all = owner * SPLITB + (srow - SPLITA)
        blocks = []
        for b in range(NBLK):
            bm = (ld >> 7) == b
            lb = ld[bm] & 127
            a_m = inA[bm]
            halves = []
            for rows_all, hm in ((idxA_all[bm], a_m), (idxB_all[bm], ~a_m)):
                rows = rows_all[hm]
                l = lb[hm]
                order = np.argsort(rows, kind="stable")
                halves.append((rows[order], l[order]))
            blocks.append(halves)
        per_core.append(blocks)

    # static per-(block, half) chunk counts = max over cores
    nch = np.zeros((NBLK, 2), dtype=np.int64)
    for b in range(NBLK):
        for r in range(2):
            mx = max(len(per_core[c][b][r][0]) for c in range(NCORE))
            nch[b, r] = max(1, -(-mx // 128))
    n16 = nch * 8  # eidx free-dim cols per gather (16 idx per col)
    eoff = [np.concatenate([[0], np.cumsum(n16[:, r])]) for r in range(2)]
    ohoff = [np.concatenate([[0], np.cumsum(nch[:, r] * 128)]) for r in range(2)]

    eidx_all = [[], []]
    ohm_all = [[], []]
    for c in range(NCORE):
        for r in range(2):
            eidx = np.zeros((128, int(eoff[r][-1])), dtype=np.int16)
            ohm = np.zeros((128, int(ohoff[r][-1])), dtype=bf16)
            for b in range(NBLK):
                rows, lb = per_core[c][b][r]
                n = len(rows)
                cap = int(nch[b, r]) * 128
                floor = (int(nch[b, r]) - 1) * 128 + 1
                ntop = max(n, min(floor, cap))  # top up with row-0 edges
                si = np.full(cap, -1, dtype=np.int16)
                si[:n] = rows.astype(np.int16)
                si[n:ntop] = 0
                e0, e1 = int(eoff[r][b]), int(eoff[r][b + 1])
                eidx[:, e0:e1] = np.tile(si.reshape(-1, 16).T, (8, 1))
                col0 = int(ohoff[r][b])
                pos = np.arange(n)
                ohm[pos % 128, col0 + (pos // 128) * 128 + lb.astype(np.int64)] = 1.0
            eidx_all[r].append(eidx)
            ohm_all[r].append(ohm)

    xts = []
    xf = np.asarray(x, dtype=np.float32)
    for c in range(NCORE):
        xt = np.zeros((DIN, NBLK * 128), dtype=bf16)
        xt[:, :NPC] = xf[c * NPC : (c + 1) * NPC].T.astype(bf16)
        xts.append(xt)

    shared = {
        "w0t": np.ascontiguousarray(np.asarray(W0, np.float32).T).astype(bf16),
        "w1t": np.ascontiguousarray(np.asarray(W1, np.float32).T).astype(bf16),
        "b0b": np.tile(np.asarray(b0, np.float32)[None, :], (128, 1)),
        "b1b": np.tile(np.asarray(b1, np.float32)[None, :], (128, 1)),
        "ar0": np.tile(
            np.asarray(att0, np.float32)[0, :, D:].reshape(-1)[None, :], (128, 1)
        ),
        "ar1": np.tile(
            np.asarray(att1, np.float32)[0, :, D:].reshape(-1)[None, :], (128, 1)
        ),
        "gmb": np.tile(np.asarray(gamma, np.float32)[None, :], (128, 1)),
        "btb": np.tile(np.asarray(beta, np.float32)[None, :], (128, 1)),
    }
    in_maps = []
    for c in range(NCORE):
        m = dict(shared)
        m["xt"] = xts[c]
        m["eidxA"] = eidx_all[0][c]
        m["eidxB"] = eidx_all[1][c]
        m["ohmA"] = ohm_all[0][c]
        m["ohmB"] = ohm_all[1][c]
        in_maps.append(m)
    return in_maps, nch, plain_ln


def _build_program(nch, plain_ln):
    """nch: [NBLK, 2] static chunk counts per (block, half)."""
    NCHA_MAX = int(nch[:, 0].max())
    NCHB_MAX = int(nch[:, 1].max())
    e16off = [np.concatenate([[0], np.cumsum(nch[:, r] * 8)]) for r in range(2)]
    ohoff = [np.concatenate([[0], np.cumsum(nch[:, r] * 128)]) for r in range(2)]
    BST0 = int(os.environ.get("KB0", "12"))  # layer-0 first B-gather step
    BST1 = int(os.environ.get("KB1", "14"))  # layer-1 first B-gather step

    nc = bacc.Bacc(
        "TRN2",
        target_bir_lowering=False,
        debug=False,
        num_devices=NCORE,
        num_swdge_queues=4,
    )

    xt_d = nc.dram_tensor("xt", [DIN, NBLK * 128], BF16, kind="ExternalInput")
    eidxA_d = nc.dram_tensor("eidxA", [128, int(e16off[0][-1])], I16, kind="ExternalInput")
    eidxB_d = nc.dram_tensor("eidxB", [128, int(e16off[1][-1])], I16, kind="ExternalInput")
    ohmA_d = nc.dram_tensor("ohmA", [128, int(ohoff[0][-1])], BF16, kind="ExternalInput")
    ohmB_d = nc.dram_tensor("ohmB", [128, int(ohoff[1][-1])], BF16, kind="ExternalInput")
    w0t_d = nc.dram_tensor("w0t", [DIN, F], BF16, kind="ExternalInput")
    w1t_d = nc.dram_tensor("w1t", [F, F], BF16, kind="ExternalInput")
    b0b_d = nc.dram_tensor("b0b", [128, F], F32, kind="ExternalInput")
    b1b_d = nc.dram_tensor("b1b", [128, F], F32, kind="ExternalInput")
    ar0_d = nc.dram_tensor("ar0", [128, F], F32, kind="ExternalInput")
    ar1_d = nc.dram_tensor("ar1", [128, F], F32, kind="ExternalInput")
    gmb_d = nc.dram_tensor("gmb", [128, F], F32, kind="ExternalInput")
    btb_d = nc.dram_tensor("btb", [128, F], F32, kind="ExternalInput")
    out_d = nc.dram_tensor("out", [NPC, D], F32, kind="ExternalOutput")
    x1_d = nc.dram_tensor("x1d", [NBLK * 128, F], BF16)

    tblA_own = [nc.dram_tensor(f"tblA_own{l}", [SPLITA, ROWE], BF16) for l in range(2)]
    tblB_own = [nc.dram_tensor(f"tblB_own{l}", [SPLITB, ROWE], BF16) for l in range(2)]
    tblA_full = [
        nc.dram_tensor(f"tblA_full{l}", [NCORE * SPLITA, ROWE], BF16, addr_space="Shared")
        for l in range(2)
    ]
    tblB_full = [
        nc.dram_tensor(f"tblB_full{l}", [NCORE * SPLITB, ROWE], BF16, addr_space="Shared")
        for l in range(2)
    ]

    groups = [list(range(NCORE))]

    with tile.TileContext(nc) as tc:
        with (
            tc.tile_pool(name="const", bufs=1) as cpool,
            tc.tile_pool(name="hbuf", bufs=3) as hpool,
            tc.tile_pool(name="small", bufs=6) as smpool,
            tc.tile_pool(name="tblt", bufs=3) as tbpool,
            tc.tile_pool(name="ohp", bufs=3) as ohpool,
            tc.tile_pool(name="post", bufs=3) as postpool,
            tc.tile_pool(name="xtp", bufs=4) as xtpool,
            tc.tile_pool(name="gemm", bufs=2, space="PSUM") as gpsum,
            tc.tile_pool(name="agga", bufs=3, space="PSUM") as apsumA,
            tc.tile_pool(name="aggb", bufs=3, space="PSUM") as apsumB,
        ):
            # ---- load constants ----
            def cload(dram, shape, dtype):
                t = cpool.tile(shape, dtype, tag=dram.name)
                nc.sync.dma_start(out=t[:], in_=dram[:, :])
                return t

            xt_s = cload(xt_d, [DIN, NBLK * 128], BF16)
            eidxA_s = cload(eidxA_d, [128, int(e16off[0][-1])], I16)
            eidxB_s = cload(eidxB_d, [128, int(e16off[1][-1])], I16)
            w0t_s = cload(w0t_d, [DIN, F], BF16)
            w1t_s = []
            for cch in range(2):
                t = cpool.tile([128, F], BF16, tag=f"w1t{cch}")
                nc.sync.dma_start(out=t[:], in_=w1t_d[cch * 128 : (cch + 1) * 128, :])
                w1t_s.append(t)
            b0b_s = cload(b0b_d, [128, F], F32)
            b1b_s = cload(b1b_d, [128, F], F32)
            ar0_s = cload(ar0_d, [128, F], F32)
            ar1_s = cload(ar1_d, [128, F], F32)
            if not plain_ln:
                gmb_s = cload(gmb_d, [128, F], F32)
                btb_s = cload(btb_d, [128, F], F32)
            epsb_s = cpool.tile([128, 1], F32, tag="epsb")
            nc.vector.memset(epsb_s[:], EPS)
            # persistent A-half accumulator, one [128, FH] f32 slab per block
            acc_s = cpool.tile([128, NBLK, FH], F32, tag="acc")

            gtA = []
            gtB = []
            for i in range(3):
                t = cpool.tile([128, NCHA_MAX, ROWE], BF16, tag=f"gtA{i}")
                nc.vector.memset(t[:], 0.0)
                gtA.append(t)
                t = cpool.tile([128, NCHB_MAX, ROWE], BF16, tag=f"gtB{i}")
                nc.vector.memset(t[:], 0.0)
                gtB.append(t)

            # hoisted num_idxs registers, one per distinct chunk count
            regs = {}
            for v in sorted(set(nch.reshape(-1).tolist())):
                regs[int(v)] = nc.gpsimd.to_reg(int(v) * 128)

            def att_scores(u, dst_ap):
                """dst = su + (K2/K1)*sa, where su/sa are +/- abs row sums of u."""
                su = smpool.tile([128, H], F32, tag="su")
                nc.vector.tensor_reduce(
                    su[:],
                    u[:].rearrange("p (h d) -> p h d", d=D),
                    axis=mybir.AxisListType.X,
                    op=ALU.add,
                )
                sa = smpool.tile([128, H], F32, tag="sa")
                nc.vector.tensor_reduce(
                    sa[:],
                    u[:].rearrange("p (h d) -> p h d", d=D),
                    axis=mybir.AxisListType.X,
                    op=ALU.add,
                    apply_absolute_value=True,
                )
                t1 = smpool.tile([128, H], F32, tag="t1")
                nc.vector.tensor_scalar(t1[:], sa[:], K2 / K1, None, op0=ALU.mult)
                nc.vector.tensor_tensor(dst_ap, su[:], t1[:], op=ALU.add)

            def emit_rows(l, t, h, srf):
                """Build [g=w*h | w] row block and DMA it to the own table."""
                rows = min(128, NPC - t * 128)
                tb = tbpool.tile([128, F + H], BF16, tag="tb")
                nc.scalar.activation(tb[:, F : F + H], srf, AF.Exp, scale=K1)
                nc.vector.tensor_tensor(
                    tb[:, 0:F].rearrange("p (h d) -> p h d", d=D),
                    h[:].rearrange("p (h d) -> p h d", d=D),
                    tb[:, F : F + H].to_broadcast((128, H, D)),
                    op=ALU.mult,
                )
                if t < NBLKA:
                    dst = tblA_own[l][t * 128 : t * 128 + rows, 0 : F + H]
                else:
                    r0 = t * 128 - SPLITA
                    dst = tblB_own[l][r0 : r0 + rows, 0 : F + H]
                nc.sync.dma_start(out=dst, in_=tb[:rows, :])

            def build0(t):
                """Layer-0 GEMM + table row for destination block t."""
                ps = gpsum.tile([128, F], F32, tag="gemm")
                nc.tensor.matmul(
                    ps[:],
                    lhsT=xt_s[:, t * 128 : (t + 1) * 128],
                    rhs=w0t_s[:],
                    start=True,
                    stop=True,
                )
                h = hpool.tile([128, F], F32, tag="h")
                nc.vector.tensor_tensor(h[:], ps[:], b0b_s[:], op=ALU.add)
                u = hpool.tile([128, F], F32, tag="u")
                nc.vector.tensor_tensor(u[:], h[:], ar0_s[:], op=ALU.mult)
                srf = smpool.tile([128, H], F32, tag="srf")
                att_scores(u, srf[:])
                emit_rows(0, t, h, srf[:])

            qctr = [0]

            def gather_half(l, t, r):
                """Issue one gather (r 0 = table A, 1 = table B) for block t."""
                qn = qctr[0] % 4
                qctr[0] += 1
                if r == 0:
                    gt, tbl, eidx = gtA[t % 3], tblA_full[l], eidxA_s
                else:
                    gt, tbl, eidx = gtB[t % 3], tblB_full[l], eidxB_s
                nchv = int(nch[t, r])
                e0 = int(e16off[r][t])
                nc.gpsimd.dma_gather(
                    gt[:, 0:nchv, :],
                    tbl[:, :],
                    eidx[:, e0 : e0 + nchv * 8],
                    nchv * 128,
                    regs[nchv],
                    ROWE,
                    single_packet=(nchv * 128 <= 1024),
                    queue_num=qn,
                )
                return gt

            def load_onehot(t, r):
                ohm = ohmA_d if r == 0 else ohmB_d
                mx = NCHA_MAX if r == 0 else NCHB_MAX
                nchv = int(nch[t, r])
                oh = ohpool.tile([128, mx * 128], BF16, tag=f"oh{r}")
                o0 = int(ohoff[r][t])
                nc.sync.dma_start(
                    out=oh[:, 0 : nchv * 128], in_=ohm[:, o0 : o0 + nchv * 128]
                )
                return oh

            def agg_matmuls(ps, gt, oh, t, r):
                nchv = int(nch[t, r])
                for b in range(nchv):
                    nc.tensor.matmul(
                        ps[:],
                        lhsT=oh[:, b * 128 : (b + 1) * 128],
                        rhs=gt[:, b, 0:FH],
                        start=(b == 0),
                        stop=(b == nchv - 1),
                    )

            ps_live = {}

            def agg_front(l, t):
                """A-half gather + matmuls, spilled to the SBUF accumulator."""
                ga = gather_half(l, t, 0)
                oh = load_onehot(t, 0)
                ps = apsumA.tile([128, FH], F32, tag="aggA")
                agg_matmuls(ps, ga, oh, t, 0)
                nc.vector.tensor_copy(acc_s[:, t, :], ps[:])

            def agg_mid(l, t):
                gb = gather_half(l, t, 1)
                oh = load_onehot(t, 1)
                ps = apsumB.tile([128, FH], F32, tag="aggB")
                agg_matmuls(ps, gb, oh, t, 1)
                ps_live[t] = ps

            def div_merge(t):
                """(accA + psB) -> a0 [128, F] and rec [128, H]."""
                ps = ps_live.pop(t)
                s = postpool.tile([128, FH], F32, tag="s")
                nc.vector.tensor_tensor(s[:], ps[:], acc_s[:, t, :], op=ALU.add)
                rec = smpool.tile([128, H], F32, tag="rec")
                nc.vector.reciprocal_approx_fast(out=rec[:], in_=s[:, F:FH])
                return s, rec

            x1T_live = {}

            def agg0_back(t):
                """num/den + LayerNorm + lrelu; write x1 and start its transpose."""
                s, rec = div_merge(t)
                a0 = postpool.tile([128, F], F32, tag="a0")
                nc.vector.tensor_tensor(
                    a0[:].rearrange("p (h d) -> p h d", d=D),
                    s[:, 0:F].rearrange("p (h d) -> p h d", d=D),
                    rec[:].to_broadcast((128, H, D)),
                    op=ALU.mult,
                )
                sm = smpool.tile([128, 1], F32, tag="sm")
                nc.vector.tensor_reduce(
                    sm[:], a0[:], axis=mybir.AxisListType.X, op=ALU.add
                )
                scr = postpool.tile([128, F], F32, tag="scr")
                sq = smpool.tile([128, 1], F32, tag="sq")
                nc.scalar.activation(scr[:], a0[:], AF.Square, accum_out=sq[:])
                mun = smpool.tile([128, 1], F32, tag="mun")
                nc.vector.tensor_scalar(mun[:], sm[:], -1.0 / F, None, op0=ALU.mult)
                m2 = smpool.tile([128, 1], F32, tag="m2")
                nc.vector.tensor_tensor(m2[:], mun[:], sm[:], op=ALU.mult)
                dv = smpool.tile([128, 1], F32, tag="dv")
                nc.vector.tensor_tensor(dv[:], sq[:], m2[:], op=ALU.add)
                rstd = smpool.tile([128, 1], F32, tag="rstd")
                nc.scalar.activation(
                    rstd[:], dv[:], AF.Abs_reciprocal_sqrt, bias=epsb_s[:], scale=1.0 / F
                )
                nmr = smpool.tile([128, 1], F32, tag="nmr")
                nc.vector.tensor_tensor(nmr[:], mun[:], rstd[:], op=ALU.mult)
                x1b = tbpool.tile([128, F], BF16, tag="x1b")
                if plain_ln:
                    nc.scalar.activation(
                        x1b[:], a0[:], AF.Lrelu, bias=nmr[:], scale=rstd[:], alpha=SLOPE
                    )
                else:
                    xn = postpool.tile([128, F], F32, tag="xn")
                    nc.scalar.activation(xn[:], a0[:], AF.Copy, bias=nmr[:], scale=rstd[:])
                    xg = postpool.tile([128, F], F32, tag="xg")
                    nc.vector.tensor_tensor(xg[:], xn[:], gmb_s[:], op=ALU.mult)
                    xgb = postpool.tile([128, F], F32, tag="xgb")
                    nc.vector.tensor_tensor(xgb[:], xg[:], btb_s[:], op=ALU.add)
                    nc.scalar.activation(x1b[:], xgb[:], AF.Lrelu, alpha=SLOPE)
                nc.sync.dma_start(out=x1_d[t * 128 : (t + 1) * 128, :], in_=x1b[:, :])
                xts = []
                for cch in range(2):
                    xtT = xtpool.tile([128, 128], BF16, tag=f"xT{cch}")
                    nc.sync.dma_start(
                        out=xtT[:],
                        in_=x1_d[t * 128 : (t + 1) * 128, cch * 128 : (cch + 1) * 128],
                        transpose=True,
                    )
                    xts.append(xtT)
                x1T_live[t] = xts

            def agg0_back2(t):
                """Layer-1 GEMM from the transposed x1 + scores + table rows."""
                xts = x1T_live.pop(t)
                ps2 = gpsum.tile([128, F], F32, tag="gemm")
                for cch in range(2):
                    nc.tensor.matmul(
                        ps2[:],
                        lhsT=xts[cch][:],
                        rhs=w1t_s[cch][:],
                        start=(cch == 0),
                        stop=(cch == 1),
                    )
                h1 = hpool.tile([128, F], F32, tag="h")
                nc.vector.tensor_tensor(h1[:], ps2[:], b1b_s[:], op=ALU.add)
                u1 = hpool.tile([128, F], F32, tag="u")
                nc.vector.tensor_tensor(u1[:], h1[:], ar1_s[:], op=ALU.mult)
                srf = smpool.tile([128, H], F32, tag="srf")
                att_scores(u1, srf[:])
                emit_rows(1, t, h1, srf[:])

            def agg1_back(t):
                """num/den then head mean for block t."""
                s, rec = div_merge(t)
                rows = min(128, NPC - t * 128)
                rec4 = smpool.tile([128, H], F32, tag="rec4")
                nc.vector.tensor_scalar(rec4[:], rec[:], 0.25, None, op0=ALU.mult)
                q = postpool.tile([128, F], F32, tag="a0")
                nc.vector.tensor_tensor(
                    q[:].rearrange("p (h d) -> p h d", d=D),
                    s[:, 0:F].rearrange("p (h d) -> p h d", d=D),
                    rec4[:].to_broadcast((128, H, D)),
                    op=ALU.mult,
                )
                p01 = postpool.tile([128, D], F32, tag="p01")
                nc.vector.tensor_tensor(p01[:], q[:, 0:D], q[:, D : 2 * D], op=ALU.add)
                p23 = postpool.tile([128, D], F32, tag="p23")
                nc.vector.tensor_tensor(
                    p23[:], q[:, 2 * D : 3 * D], q[:, 3 * D : 4 * D], op=ALU.add
                )
                o = postpool.tile([128, D], F32, tag="o")
                nc.vector.tensor_tensor(o[:], p01[:], p23[:], op=ALU.add)
                nc.sync.dma_start(
                    out=out_d[t * 128 : t * 128 + rows, :], in_=o[:rows, :]
                )

            def allgather(l, half):
                own = tblA_own[l] if half == 0 else tblB_own[l]
                full = tblA_full[l] if half == 0 else tblB_full[l]
                nc.gpsimd.collective_compute(
                    "AllGather",
                    ALU.bypass,
                    replica_groups=groups,
                    ins=[own[:, :]],
                    outs=[full[:, :]],
                )

            # ================= schedule =================
            def mid_steps(bstart):
                """mid step per block: catch-up (2 mids/step) from bstart."""
                ms = {}
                avail = bstart
                for t in range(NBLK):
                    s = max(t + 2, avail)
                    ms.setdefault(s, []).append(t)
                    avail = s + 1 if len(ms[s]) >= 2 else s
                return ms

            def agg_sweep(l, back, back2, bstart, post_a=None, post_all=None):
                ms = mid_steps(bstart)
                last_step = max(ms) + (2 if back2 else 0)
                for step in range(0, last_step + 1):
                    for t in ms.get(step, []):
                        agg_mid(l, t)
                    # back2 first: its inputs are 2 steps old, so it gives the
                    # DVE/PE queues work while back() waits on the B PSUM stop.
                    if back2 is not None:
                        for t in ms.get(step - 2, []):
                            back2(t)
                            if post_a is not None and t == NBLKA - 1:
                                post_a()
                            if post_all is not None and t == NBLK - 1:
                                post_all()
                    for t in ms.get(step, []):
                        back(t)
                    if step < NBLK:
                        agg_front(l, step)

            # ---- layer 0 build ----
            for t in range(NBLKA):
                build0(t)
            allgather(0, 0)
            for t in range(NBLKA, NBLK):
                build0(t)
            allgather(0, 1)
            # ---- layer 0 aggregate + layer 1 build (fused) ----
            agg_sweep(
                0,
                agg0_back,
                agg0_back2,
                BST0,
                post_a=lambda: allgather(1, 0),
                post_all=lambda: allgather(1, 1),
            )
            # ---- layer 1 aggregate ----
            agg_sweep(1, agg1_back, None, BST1)

    nc.compile()
    return nc


_CACHE = {}


def kernel(**inputs):
    global LAST_RESULTS
    in_maps, nch, plain_ln = _host_prep(**inputs)
    key = (
        tuple(nch.reshape(-1).tolist()),
        plain_ln,
        os.environ.get("KB0"),
        os.environ.get("KB1"),
    )
    if key not in _CACHE:
        _CACHE[key] = _build_program(nch, plain_ln)
    nc = _CACHE[key]
    trace = bool(os.environ.get("BASS_TRACE"))
    res = run_bass_kernel_spmd(nc, in_maps, list(range(NCORE)), trace=trace)
    LAST_RESULTS = res
    out = np.concatenate([res.results[c]["out"] for c in range(NCORE)], axis=0)
    return out.astype(np.float32)


# revision 9
# speedup vs baseline: 1.2430x; 1.2430x over previous
"""GAT-style 2-layer GNN message passing on 8 Trainium2 NeuronCores.

Math note: for this reference, the segment-softmax ratio
  num/den = (sum_j h[j]*exp((s_l[i]+s_r[j])/2d)) / (sum_j exp((s_l[i]+s_r[j])/2d))
has the destination factor exp(s_l[i]/2d) cancel, so per layer we only need
  a[i] = (sum_{j in N(i)} w_j*h_j) / (sum_{j in N(i)} w_j),  w_j = exp(s_r[j]/2d).

Sharding: nodes split into 8 contiguous destination ranges (6250/core).
Each core builds table rows [g=w*h (256) | w (4) | pad] (bf16, 768B) for its
own nodes; the table is split in two halves by local row (A: 0..3199,
B: 3200..6249) and distributed with two pipelined AllGathers per layer.
Each core aggregates its own destinations: per-edge dma_gather of source
rows (HBM row order sorted ascending per gather for locality, per-block
chunk-exact counts with -1 tail trim), then one-hot matmul segment-sum into
PSUM. A-half partial sums spill to an SBUF accumulator so the A stream can
run ahead while the B-half AllGather is still in flight; B-half partials
merge with the accumulator at divide time. Layer-1 table rows are built
inline in the layer-0 aggregation sweep (the x1 transpose for the layer-1
GEMM goes through a DRAM DMA-transpose round trip).
"""

import os
import sys

import numpy as np
import ml_dtypes

sys.path.insert(0, "/opt/trn_rl_repo")

import concourse.bacc as bacc
import concourse.bass as bass
import concourse.mybir as mybir
import concourse.tile as tile
from concourse.bass_utils import run_bass_kernel_spmd

BF16 = mybir.dt.bfloat16
F32 = mybir.dt.float32
I16 = mybir.dt.int16

N, DIN, E = 50000, 128, 800000
H, D = 4, 64
F = H * D  # 256
FH = F + H  # 260
NCORE = 8
NPC = N // NCORE  # 6250
NBLK = (NPC + 127) // 128  # 49 destination blocks per core
EPS = 1e-5
SLOPE = 0.01
ROWE = 384  # table row: 256 g + 4 w + 124 pad (bf16) = 768 bytes
SPLITA = 3200  # local rows 0..3199 -> table A (25 blocks), rest -> table B
SPLITB = NPC - SPLITA  # 3050
NBLKA = SPLITA // 128  # 25
K1 = (1 + SLOPE) / 2 / (2 * D)
K2 = (1 - SLOPE) / 2 / (2 * D)

LAST_RESULTS = None

AF = mybir.ActivationFunctionType
ALU = mybir.AluOpType


def _host_prep(x, edge_index, W0, b0, W1, b1, att0, att1, gamma, beta):
    """Build all per-core and shared input arrays.

    Per (core, block, half) the edge list is sorted by source table row.
    All cores share one static chunk count per (block, half): the max over
    cores. Cores short of (nch-1)*128+1 edges are topped up with row-0
    dummy edges (one-hot zero), then -1-padded to nch*128 so the gather
    ucode trims the tail to the top-up count on every core (keeping the
    descriptor count consistent with the ring reservation).
    """
    bf16 = ml_dtypes.bfloat16
    dst = np.asarray(edge_index[0], dtype=np.int64)
    src = np.asarray(edge_index[1], dtype=np.int64)

    plain_ln = bool(
        np.allclose(np.asarray(gamma), 1.0) and np.allclose(np.asarray(beta), 0.0)
    )

    per_core = []  # [c][b][half] -> (rows_sorted, lb_sorted)
    for c in range(NCORE):
        m = (dst >= c * NPC) & (dst < (c + 1) * NPC)
        ld = dst[m] - c * NPC
        s = src[m]
        owner = s // NPC
        srow = s - owner * NPC
        inA = srow < SPLITA
        idxA_all = owner * SPLITA + srow
        idxB_all = owner * SPLITB + (srow - SPLITA)
        blocks = []
        for b in range(NBLK):
            bm = (ld >> 7) == b
            lb = ld[bm] & 127
            a_m = inA[bm]
            halves = []
            for rows_all, hm in ((idxA_all[bm], a_m), (idxB_all[bm], ~a_m)):
                rows = rows_all[hm]
                l = lb[hm]
                if os.environ.get("KSORT", "0") == "1":
                    order = np.argsort(rows, kind="stable")
                    rows, l = rows[order], l[order]
                halves.append((rows, l))
            blocks.append(halves)
        per_core.append(blocks)

    # static per-(block, half) chunk counts = max over cores
    nch = np.zeros((NBLK, 2), dtype=np.int64)
    for b in range(NBLK):
        for r in range(2):
            mx = max(len(per_core[c][b][r][0]) for c in range(NCORE))
            nch[b, r] = max(1, -(-mx // 128))
    n16 = nch * 8  # eidx free-dim cols per gather (16 idx per col)
    eoff = [np.concatenate([[0], np.cumsum(n16[:, r])]) for r in range(2)]
    ohoff = [np.concatenate([[0], np.cumsum(nch[:, r] * 128)]) for r in range(2)]

    eidx_all = [[], []]
    ohm_all = [[], []]
    for c in range(NCORE):
        for r in range(2):
            eidx = np.zeros((128, int(eoff[r][-1])), dtype=np.int16)
            ohm = np.zeros((128, int(ohoff[r][-1])), dtype=bf16)
            for b in range(NBLK):
                rows, lb = per_core[c][b][r]
                n = len(rows)
                cap = int(nch[b, r]) * 128
                floor = (int(nch[b, r]) - 1) * 128 + 1
                ntop = max(n, min(floor, cap))  # top up with row-0 edges
                si = np.full(cap, -1, dtype=np.int16)
                si[:n] = rows.astype(np.int16)
                si[n:ntop] = 0
                e0, e1 = int(eoff[r][b]), int(eoff[r][b + 1])
                eidx[:, e0:e1] = np.tile(si.reshape(-1, 16).T, (8, 1))
                col0 = int(ohoff[r][b])
                pos = np.arange(n)
                ohm[pos % 128, col0 + (pos // 128) * 128 + lb.astype(np.int64)] = 1.0
            eidx_all[r].append(eidx)
            ohm_all[r].append(ohm)

    xts = []
    xf = np.asarray(x, dtype=np.float32)
    for c in range(NCORE):
        xt = np.zeros((DIN, NBLK * 128), dtype=bf16)
        xt[:, :NPC] = xf[c * NPC : (c + 1) * NPC].T.astype(bf16)
        xts.append(xt)

    shared = {
        "w0t": np.ascontiguousarray(np.asarray(W0, np.float32).T).astype(bf16),
        "w1t": np.ascontiguousarray(np.asarray(W1, np.float32).T).astype(bf16),
        "b0b": np.tile(np.asarray(b0, np.float32)[None, :], (128, 1)),
        "b1b": np.tile(np.asarray(b1, np.float32)[None, :], (128, 1)),
        "ar0": np.tile(
            np.asarray(att0, np.float32)[0, :, D:].reshape(-1)[None, :], (128, 1)
        ),
        "ar1": np.tile(
            np.asarray(att1, np.float32)[0, :, D:].reshape(-1)[None, :], (128, 1)
        ),
        "gmb": np.tile(np.asarray(gamma, np.float32)[None, :], (128, 1)),
        "btb": np.tile(np.asarray(beta, np.float32)[None, :], (128, 1)),
        "idn": np.eye(128, dtype=np.float32).astype(bf16),
    }
    in_maps = []
    for c in range(NCORE):
        m = dict(shared)
        m["xt"] = xts[c]
        m["eidxA"] = eidx_all[0][c]
        m["eidxB"] = eidx_all[1][c]
        m["ohmA"] = ohm_all[0][c]
        m["ohmB"] = ohm_all[1][c]
        in_maps.append(m)
    return in_maps, nch, plain_ln


def _build_program(nch, plain_ln):
    """nch: [NBLK, 2] static chunk counts per (block, half)."""
    NCHA_MAX = int(nch[:, 0].max())
    NCHB_MAX = int(nch[:, 1].max())
    e16off = [np.concatenate([[0], np.cumsum(nch[:, r] * 8)]) for r in range(2)]
    ohoff = [np.concatenate([[0], np.cumsum(nch[:, r] * 128)]) for r in range(2)]
    BST0 = int(os.environ.get("KB0", "12"))  # layer-0 first B-gather step
    BST1 = int(os.environ.get("KB1", "14"))  # layer-1 first B-gather step

    nc = bacc.Bacc(
        "TRN2",
        target_bir_lowering=False,
        debug=False,
        num_devices=NCORE,
        num_swdge_queues=4,
    )

    xt_d = nc.dram_tensor("xt", [DIN, NBLK * 128], BF16, kind="ExternalInput")
    eidxA_d = nc.dram_tensor("eidxA", [128, int(e16off[0][-1])], I16, kind="ExternalInput")
    eidxB_d = nc.dram_tensor("eidxB", [128, int(e16off[1][-1])], I16, kind="ExternalInput")
    ohmA_d = nc.dram_tensor("ohmA", [128, int(ohoff[0][-1])], BF16, kind="ExternalInput")
    ohmB_d = nc.dram_tensor("ohmB", [128, int(ohoff[1][-1])], BF16, kind="ExternalInput")
    w0t_d = nc.dram_tensor("w0t", [DIN, F], BF16, kind="ExternalInput")
    w1t_d = nc.dram_tensor("w1t", [F, F], BF16, kind="ExternalInput")
    b0b_d = nc.dram_tensor("b0b", [128, F], F32, kind="ExternalInput")
    b1b_d = nc.dram_tensor("b1b", [128, F], F32, kind="ExternalInput")
    ar0_d = nc.dram_tensor("ar0", [128, F], F32, kind="ExternalInput")
    ar1_d = nc.dram_tensor("ar1", [128, F], F32, kind="ExternalInput")
    gmb_d = nc.dram_tensor("gmb", [128, F], F32, kind="ExternalInput")
    btb_d = nc.dram_tensor("btb", [128, F], F32, kind="ExternalInput")
    idn_d = nc.dram_tensor("idn", [128, 128], BF16, kind="ExternalInput")
    out_d = nc.dram_tensor("out", [NPC, D], F32, kind="ExternalOutput")

    tblA_own = [nc.dram_tensor(f"tblA_own{l}", [SPLITA, ROWE], BF16) for l in range(2)]
    tblB_own = [nc.dram_tensor(f"tblB_own{l}", [SPLITB, ROWE], BF16) for l in range(2)]
    tblA_full = [
        nc.dram_tensor(f"tblA_full{l}", [NCORE * SPLITA, ROWE], BF16, addr_space="Shared")
        for l in range(2)
    ]
    tblB_full = [
        nc.dram_tensor(f"tblB_full{l}", [NCORE * SPLITB, ROWE], BF16, addr_space="Shared")
        for l in range(2)
    ]

    groups = [list(range(NCORE))]

    with tile.TileContext(nc) as tc:
        with (
            tc.tile_pool(name="const", bufs=1) as cpool,
            tc.tile_pool(name="hbuf", bufs=3) as hpool,
            tc.tile_pool(name="small", bufs=6) as smpool,
            tc.tile_pool(name="tblt", bufs=3) as tbpool,
            tc.tile_pool(name="ohp", bufs=3) as ohpool,
            tc.tile_pool(name="post", bufs=3) as postpool,
            tc.tile_pool(name="xtp", bufs=4) as xtpool,
            tc.tile_pool(name="gemm", bufs=1, space="PSUM") as gpsum,
            tc.tile_pool(name="agga", bufs=3, space="PSUM") as apsumA,
            tc.tile_pool(name="aggb", bufs=2, space="PSUM") as apsumB,
            tc.tile_pool(name="tp", bufs=1, space="PSUM") as tpsum,
        ):
            # ---- load constants ----
            def cload(dram, shape, dtype):
                t = cpool.tile(shape, dtype, tag=dram.name)
                nc.sync.dma_start(out=t[:], in_=dram[:, :])
                return t

            xt_s = cload(xt_d, [DIN, NBLK * 128], BF16)
            eidxA_s = cload(eidxA_d, [128, int(e16off[0][-1])], I16)
            eidxB_s = cload(eidxB_d, [128, int(e16off[1][-1])], I16)
            w0t_s = cload(w0t_d, [DIN, F], BF16)
            w1t_s = []
            for cch in range(2):
                t = cpool.tile([128, F], BF16, tag=f"w1t{cch}")
                nc.sync.dma_start(out=t[:], in_=w1t_d[cch * 128 : (cch + 1) * 128, :])
                w1t_s.append(t)
            b0b_s = cload(b0b_d, [128, F], F32)
            b1b_s = cload(b1b_d, [128, F], F32)
            ar0_s = cload(ar0_d, [128, F], F32)
            ar1_s = cload(ar1_d, [128, F], F32)
            if not plain_ln:
                gmb_s = cload(gmb_d, [128, F], F32)
                btb_s = cload(btb_d, [128, F], F32)
            idn_s = cload(idn_d, [128, 128], BF16)
            epsb_s = cpool.tile([128, 1], F32, tag="epsb")
            nc.vector.memset(epsb_s[:], EPS)
            # persistent A-half accumulator, one [128, FH] f32 slab per block
            acc_s = cpool.tile([128, NBLK, FH], F32, tag="acc")

            gtA = []
            gtB = []
            for i in range(3):
                t = cpool.tile([128, NCHA_MAX, ROWE], BF16, tag=f"gtA{i}")
                nc.vector.memset(t[:], 0.0)
                gtA.append(t)
                t = cpool.tile([128, NCHB_MAX, ROWE], BF16, tag=f"gtB{i}")
                nc.vector.memset(t[:], 0.0)
                gtB.append(t)

            # hoisted num_idxs registers, one per distinct chunk count
            regs = {}
            for v in sorted(set(nch.reshape(-1).tolist())):
                regs[int(v)] = nc.gpsimd.to_reg(int(v) * 128)

            def att_scores(u, dst_ap):
                """dst = su + (K2/K1)*sa, where su/sa are +/- abs row sums of u."""
                su = smpool.tile([128, H], F32, tag="su")
                nc.vector.tensor_reduce(
                    su[:],
                    u[:].rearrange("p (h d) -> p h d", d=D),
                    axis=mybir.AxisListType.X,
                    op=ALU.add,
                )
                sa = smpool.tile([128, H], F32, tag="sa")
                nc.vector.tensor_reduce(
                    sa[:],
                    u[:].rearrange("p (h d) -> p h d", d=D),
                    axis=mybir.AxisListType.X,
                    op=ALU.add,
                    apply_absolute_value=True,
                )
                t1 = smpool.tile([128, H], F32, tag="t1")
                nc.vector.tensor_scalar(t1[:], sa[:], K2 / K1, None, op0=ALU.mult)
                nc.vector.tensor_tensor(dst_ap, su[:], t1[:], op=ALU.add)

            def emit_rows(l, t, h, srf):
                """Build [g=w*h | w] row block and DMA it to the own table."""
                rows = min(128, NPC - t * 128)
                tb = tbpool.tile([128, F + H], BF16, tag="tb")
                nc.scalar.activation(tb[:, F : F + H], srf, AF.Exp, scale=K1)
                nc.vector.tensor_tensor(
                    tb[:, 0:F].rearrange("p (h d) -> p h d", d=D),
                    h[:].rearrange("p (h d) -> p h d", d=D),
                    tb[:, F : F + H].to_broadcast((128, H, D)),
                    op=ALU.mult,
                )
                if t < NBLKA:
                    dst = tblA_own[l][t * 128 : t * 128 + rows, 0 : F + H]
                else:
                    r0 = t * 128 - SPLITA
                    dst = tblB_own[l][r0 : r0 + rows, 0 : F + H]
                nc.sync.dma_start(out=dst, in_=tb[:rows, :])

            def build0(t):
                """Layer-0 GEMM + table row for destination block t."""
                ps = gpsum.tile([128, F], F32, tag="gemm")
                nc.tensor.matmul(
                    ps[:],
                    lhsT=xt_s[:, t * 128 : (t + 1) * 128],
                    rhs=w0t_s[:],
                    start=True,
                    stop=True,
                )
                h = hpool.tile([128, F], F32, tag="h")
                nc.vector.tensor_tensor(h[:], ps[:], b0b_s[:], op=ALU.add)
                u = hpool.tile([128, F], F32, tag="u")
                nc.vector.tensor_tensor(u[:], h[:], ar0_s[:], op=ALU.mult)
                srf = smpool.tile([128, H], F32, tag="srf")
                att_scores(u, srf[:])
                emit_rows(0, t, h, srf[:])

            qctr = [0]

            def gather_half(l, t, r):
                """Issue one gather (r 0 = table A, 1 = table B) for block t."""
                qn = qctr[0] % 4
                qctr[0] += 1
                if r == 0:
                    gt, tbl, eidx = gtA[t % 3], tblA_full[l], eidxA_s
                else:
                    gt, tbl, eidx = gtB[t % 3], tblB_full[l], eidxB_s
                nchv = int(nch[t, r])
                e0 = int(e16off[r][t])
                nc.gpsimd.dma_gather(
                    gt[:, 0:nchv, :],
                    tbl[:, :],
                    eidx[:, e0 : e0 + nchv * 8],
                    nchv * 128,
                    regs[nchv],
                    ROWE,
                    single_packet=(nchv * 128 <= 1024),
                    queue_num=qn,
                )
                return gt

            def load_onehot(t, r):
                ohm = ohmA_d if r == 0 else ohmB_d
                mx = NCHA_MAX if r == 0 else NCHB_MAX
                nchv = int(nch[t, r])
                oh = ohpool.tile([128, mx * 128], BF16, tag=f"oh{r}")
                o0 = int(ohoff[r][t])
                nc.sync.dma_start(
                    out=oh[:, 0 : nchv * 128], in_=ohm[:, o0 : o0 + nchv * 128]
                )
                return oh

            def agg_matmuls(ps, gt, oh, t, r):
                nchv = int(nch[t, r])
                for b in range(nchv):
                    nc.tensor.matmul(
                        ps[:],
                        lhsT=oh[:, b * 128 : (b + 1) * 128],
                        rhs=gt[:, b, 0:FH],
                        start=(b == 0),
                        stop=(b == nchv - 1),
                    )

            ps_live = {}

            def agg_front(l, t):
                """A-half gather + matmuls, spilled to the SBUF accumulator."""
                ga = gather_half(l, t, 0)
                oh = load_onehot(t, 0)
                ps = apsumA.tile([128, FH], F32, tag="aggA")
                agg_matmuls(ps, ga, oh, t, 0)
                nc.vector.tensor_copy(acc_s[:, t, :], ps[:])

            def agg_mid(l, t):
                gb = gather_half(l, t, 1)
                oh = load_onehot(t, 1)
                ps = apsumB.tile([128, FH], F32, tag="aggB")
                agg_matmuls(ps, gb, oh, t, 1)
                ps_live[t] = ps

            def div_merge(t):
                """(accA + psB) -> a0 [128, F] and rec [128, H]."""
                ps = ps_live.pop(t)
                s = postpool.tile([128, FH], F32, tag="s")
                nc.vector.tensor_tensor(s[:], ps[:], acc_s[:, t, :], op=ALU.add)
                rec = smpool.tile([128, H], F32, tag="rec")
                nc.vector.reciprocal_approx_fast(out=rec[:], in_=s[:, F:FH])
                return s, rec

            x1T_live = {}

            def agg0_back(t):
                """num/den + LayerNorm + lrelu; write x1 and start its transpose."""
                s, rec = div_merge(t)
                a0 = postpool.tile([128, F], F32, tag="a0")
                nc.vector.tensor_tensor(
                    a0[:].rearrange("p (h d) -> p h d", d=D),
                    s[:, 0:F].rearrange("p (h d) -> p h d", d=D),
                    rec[:].to_broadcast((128, H, D)),
                    op=ALU.mult,
                )
                sm = smpool.tile([128, 1], F32, tag="sm")
                nc.vector.tensor_reduce(
                    sm[:], a0[:], axis=mybir.AxisListType.X, op=ALU.add
                )
                scr = postpool.tile([128, F], F32, tag="scr")
                nc.vector.tensor_tensor(scr[:], a0[:], a0[:], op=ALU.mult)
                sq = smpool.tile([128, 1], F32, tag="sq")
                nc.vector.tensor_reduce(
                    sq[:], scr[:], axis=mybir.AxisListType.X, op=ALU.add
                )
                mun = smpool.tile([128, 1], F32, tag="mun")
                nc.vector.tensor_scalar(mun[:], sm[:], -1.0 / F, None, op0=ALU.mult)
                m2 = smpool.tile([128, 1], F32, tag="m2")
                nc.vector.tensor_tensor(m2[:], mun[:], sm[:], op=ALU.mult)
                dv = smpool.tile([128, 1], F32, tag="dv")
                nc.vector.tensor_tensor(dv[:], sq[:], m2[:], op=ALU.add)
                rstd = smpool.tile([128, 1], F32, tag="rstd")
                nc.scalar.activation(
                    rstd[:], dv[:], AF.Abs_reciprocal_sqrt, bias=epsb_s[:], scale=1.0 / F
                )
                nmr = smpool.tile([128, 1], F32, tag="nmr")
                nc.vector.tensor_tensor(nmr[:], mun[:], rstd[:], op=ALU.mult)
                x1b = tbpool.tile([128, F], BF16, tag="x1b")
                if plain_ln:
                    nc.scalar.activation(
                        x1b[:], a0[:], AF.Lrelu, bias=nmr[:], scale=rstd[:], alpha=SLOPE
                    )
                else:
                    xn = postpool.tile([128, F], F32, tag="xn")
                    nc.scalar.activation(xn[:], a0[:], AF.Copy, bias=nmr[:], scale=rstd[:])
                    xg = postpool.tile([128, F], F32, tag="xg")
                    nc.vector.tensor_tensor(xg[:], xn[:], gmb_s[:], op=ALU.mult)
                    xgb = postpool.tile([128, F], F32, tag="xgb")
                    nc.vector.tensor_tensor(xgb[:], xg[:], btb_s[:], op=ALU.add)
                    nc.scalar.activation(x1b[:], xgb[:], AF.Lrelu, alpha=SLOPE)
                xts = []
                for cch in range(2):
                    pt = tpsum.tile([128, 128], BF16, tag=f"tp{cch}")
                    nc.tensor.transpose(
                        pt[:], x1b[:, cch * 128 : (cch + 1) * 128], idn_s[:]
                    )
                    xtT = xtpool.tile([128, 128], BF16, tag=f"xT{cch}")
                    nc.vector.tensor_copy(xtT[:], pt[:])
                    xts.append(xtT)
                x1T_live[t] = xts

            def agg0_back2(t):
                """Layer-1 GEMM from the transposed x1 + scores + table rows."""
                xts = x1T_live.pop(t)
                ps2 = gpsum.tile([128, F], F32, tag="gemm")
                for cch in range(2):
                    nc.tensor.matmul(
                        ps2[:],
                        lhsT=xts[cch][:],
                        rhs=w1t_s[cch][:],
                        start=(cch == 0),
                        stop=(cch == 1),
                    )
                h1 = hpool.tile([128, F], F32, tag="h")
                nc.vector.tensor_tensor(h1[:], ps2[:], b1b_s[:], op=ALU.add)
                u1 = hpool.tile([128, F], F32, tag="u")
                nc.vector.tensor_tensor(u1[:], h1[:], ar1_s[:], op=ALU.mult)
                srf = smpool.tile([128, H], F32, tag="srf")
                att_scores(u1, srf[:])
                emit_rows(1, t, h1, srf[:])

            def agg1_back(t):
                """num/den then head mean for block t."""
                s, rec = div_merge(t)
                rows = min(128, NPC - t * 128)
                rec4 = smpool.tile([128, H], F32, tag="rec4")
                nc.vector.tensor_scalar(rec4[:], rec[:], 0.25, None, op0=ALU.mult)
                q = postpool.tile([128, F], F32, tag="a0")
                nc.vector.tensor_tensor(
                    q[:].rearrange("p (h d) -> p h d", d=D),
                    s[:, 0:F].rearrange("p (h d) -> p h d", d=D),
                    rec4[:].to_broadcast((128, H, D)),
                    op=ALU.mult,
                )
                p01 = postpool.tile([128, D], F32, tag="p01")
                nc.vector.tensor_tensor(p01[:], q[:, 0:D], q[:, D : 2 * D], op=ALU.add)
                p23 = postpool.tile([128, D], F32, tag="p23")
                nc.vector.tensor_tensor(
                    p23[:], q[:, 2 * D : 3 * D], q[:, 3 * D : 4 * D], op=ALU.add
                )
                o = postpool.tile([128, D], F32, tag="o")
                nc.vector.tensor_tensor(o[:], p01[:], p23[:], op=ALU.add)
                nc.sync.dma_start(
                    out=out_d[t * 128 : t * 128 + rows, :], in_=o[:rows, :]
                )

            def allgather(l, half):
                own = tblA_own[l] if half == 0 else tblB_own[l]
                full = tblA_full[l] if half == 0 else tblB_full[l]
                nc.gpsimd.collective_compute(
                    "AllGather",
                    ALU.bypass,
                    replica_groups=groups,
                    ins=[own[:, :]],
                    outs=[full[:, :]],
                )

            # ================= schedule =================
            def mid_steps(bstart):
                """mid step per block: catch-up (2 mids/step) from bstart."""
                ms = {}
                avail = bstart
                for t in range(NBLK):
                    s = max(t + 2, avail)
                    ms.setdefault(s, []).append(t)
                    avail = s + 1 if len(ms[s]) >= 2 else s
                return ms

            def agg_sweep(l, back, back2, bstart, post_a=None, post_all=None):
                ms = mid_steps(bstart)
                last_step = max(ms) + (2 if back2 else 0)
                for step in range(0, last_step + 1):
                    for t in ms.get(step, []):
                        agg_mid(l, t)
                    # back2 first: its inputs are 2 steps old, so it gives the
                    # DVE/PE queues work while back() waits on the B PSUM stop.
                    if back2 is not None:
                        for t in ms.get(step - 2, []):
                            back2(t)
                            if post_a is not None and t == NBLKA - 1:
                                post_a()
                            if post_all is not None and t == NBLK - 1:
                                post_all()
                    for t in ms.get(step, []):
                        back(t)
                    if step < NBLK:
                        agg_front(l, step)

            # ---- layer 0 build ----
            for t in range(NBLKA):
                build0(t)
            allgather(0, 0)
            for t in range(NBLKA, NBLK):
                build0(t)
            allgather(0, 1)
            # ---- layer 0 aggregate + layer 1 build (fused) ----
            agg_sweep(
                0,
                agg0_back,
                agg0_back2,
                BST0,
                post_a=lambda: allgather(1, 0),
                post_all=lambda: allgather(1, 1),
            )
            # ---- layer 1 aggregate ----
            agg_sweep(1, agg1_back, None, BST1)

    nc.compile()
    return nc


_CACHE = {}


def kernel(**inputs):
    global LAST_RESULTS
    in_maps, nch, plain_ln = _host_prep(**inputs)
    key = (
        tuple(nch.reshape(-1).tolist()),
        plain_ln,
        os.environ.get("KB0"),
        os.environ.get("KB1"),
        os.environ.get("KSORT"),
    )
    if key not in _CACHE:
        _CACHE[key] = _build_program(nch, plain_ln)
    nc = _CACHE[key]
    trace = bool(os.environ.get("BASS_TRACE"))
    res = run_bass_kernel_spmd(nc, in_maps, list(range(NCORE)), trace=trace)
    LAST_RESULTS = res
    out = np.concatenate([res.results[c]["out"] for c in range(NCORE)], axis=0)
    return out.astype(np.float32)
